# revision 1
# baseline (speedup 1.0000x reference)
"""Linear attention (B=4, S=4096, D=1024, H=16) on 8 TRN2 NeuronCores.

Sharding: core = (batch, head-half): each core handles one batch's 8 heads.
 - x is host-transposed to xT [D, S] per batch so both operand orientations
   of every matmul come out of the tensor engine with no on-device transpose.
 - Wqkv column-sharded per head-half; Wo row-sharded; host sums the two
   partial y's per batch (row-parallel unshard).

Per-core dataflow (S=4096 in 8 blocks of 512 tokens), all matmuls bf16
(fp32 PSUM accumulate; x/Wqkv/Wo host-cast to bf16):
  phase A: qkv projection:
      QT [512f, S] feature-major  (lhsT=Wq, rhs=xT)   -> elu+1 -> bf16 QT
      K,V [S, 512f] token-major   (lhsT=xT, rhs=Wkv)  -> elu+1(K), copy(V)
      (elu(x)+1 = min(exp(x),1) + relu(x): ACT Exp + DVE max + DVE stt)
  phase B: per head-pair [KV | K_sum^T] PSUM accumulation over all tokens
      (vst carries a ones column per pair so one matmul does both)
  phase C (pipelined with D of the previous block), all pair-packed:
      psc[128,s] = blockdiag(KV_h0, KV_h1)^T @ QT_pair -> both heads in one
      matmul, one ACT evict to outu, halves aligned for the apply
      norm rows via zero-padded M=32 pair matmuls -> one PSUM bank, rows
      32k/32k+1; rcp = exp(-ln(norm+eps)) (2 full-bank ACT ops; DVE
      reciprocal is 1-lane iterative and 25x slower here)
      rcpb: one M=128 matmul per pair against a block-structured ones2
      operand broadcasts both heads' rcp rows to the matching halves
      outT = outu * rcpb (one full-width DVE op per pair, bf16)
  phase D: y[s,:] (+)= outT^T @ Wo  (bf16 matmul, fp32 out, skewed one
      block behind C so the PE never waits on the normalizer chain)
"""

import numpy as np

import concourse.bacc as bacc
import concourse.mybir as mybir
import concourse.tile as tile
from concourse.bass_utils import run_bass_kernel_spmd

F32 = mybir.dt.float32
F32R = mybir.dt.float32r
BF16 = mybir.dt.bfloat16

P = 128
B, S, D = 4, 4096, 1024
H = 16
HD = 64
EPS = 1e-6

FSH = 512            # features per core for each of Q, K, V (8 heads)
KSUB = D // P        # 8 contraction subtiles
SBLK = 512           # tokens per block
NBLK = S // SBLK     # 8 blocks
TSUB = SBLK // P     # 4 token subtiles per block
NPAIR = 4            # head pairs per core
NHEAD = 8            # heads per core

_NC_CACHE = None


def build():
    nc = bacc.Bacc(target_bir_lowering=False)
    xT = nc.dram_tensor("xT", [D, S], BF16, kind="ExternalInput")
    wqkv = nc.dram_tensor("wqkv", [D, 3 * FSH], BF16, kind="ExternalInput")
    wo = nc.dram_tensor("wo", [FSH, D], BF16, kind="ExternalInput")
    ones2 = nc.dram_tensor("ones2", [P, P], F32R, kind="ExternalInput")
    y = nc.dram_tensor("y", [S, D], F32, kind="ExternalOutput")

    xT_r = xT.rearrange("(ko p) s -> p ko s", p=P)        # [128, 8, 4096]
    wqkv_r = wqkv.rearrange("(ko p) f -> p ko f", p=P)    # [128, 8, 1536]
    wo_r = wo.rearrange("(fo p) n -> p fo n", p=P)        # [128, 4, 1024]
    y_r2 = y.rearrange(
        "(j th t p) n -> j p th t n", th=TSUB // 2, t=2, p=P
    )  # [8, 128, 2, 2, 1024]

    with tile.TileContext(nc) as tc:
        import contextlib

        with contextlib.ExitStack() as ctx:
            const = ctx.enter_context(tc.tile_pool(name="const", bufs=1))
            wpool = ctx.enter_context(tc.tile_pool(name="wpool", bufs=1))
            qtpool = ctx.enter_context(tc.tile_pool(name="qtpool", bufs=1))

            # persistent SBUF
            wqkv_sb = wpool.tile([P, KSUB, 3 * FSH], BF16)
            nc.sync.dma_start(out=wqkv_sb, in_=wqkv_r)
            wo_sb = wpool.tile([P, FSH // P, D], BF16)
            nc.sync.dma_start(out=wo_sb, in_=wo_r)
            qt_sb = qtpool.tile([P, FSH // P, S], BF16)   # feature-major Q
            # per-pair block-diagonal [[KV_h0, 0], [0, KV_h1]] (128x128):
            # one matmul against the stacked QT pair computes both heads
            lhsT2_sb = [
                qtpool.tile([P, P], BF16, name=f"lhsT2{p}") for p in range(NPAIR)
            ]
            # per-pair [Ksum_h0 | Ksum_h1 | zeros] (128 x 32): col 0 rows 0:64
            # = Ksum_even, col 1 rows 64:128 = Ksum_odd
            ksumpad_sb = [
                qtpool.tile([P, 32], BF16, name=f"ksp{p}") for p in range(NPAIR)
            ]
            # norm-path scratch (only partition row 64 is used; one buf each)

            eps_sb = const.tile([P, 1], F32)
            nc.vector.memset(eps_sb, EPS)
            # ones2 (host-built): per 32-block, row 32k = [1x64 | 0x64],
            # row 32k+1 = [0x64 | 1x64] -- pair-broadcast stationary operand
            ones2_fr = const.tile([P, P], F32R)
            nc.sync.dma_start(out=ones2_fr, in_=ones2[:])

            # ---------------- phase A + B ----------------
            with (
                tc.tile_pool(name="kvps", bufs=1, space="PSUM") as kvps_pool,
                tc.tile_pool(name="xin", bufs=3) as xpool,
                tc.tile_pool(name="stage", bufs=3) as stpool,
                tc.tile_pool(name="paps", bufs=4, space="PSUM") as pa_ps,
                tc.tile_pool(name="etmp", bufs=4) as etpool,
            ):
                kvps = [
                    kvps_pool.tile([P, P + 1], F32, tag=f"kv{p}", name=f"kv{p}")
                    for p in range(NPAIR)
                ]

                for j in range(NBLK):
                    xt = xpool.tile([P, KSUB, SBLK], BF16, tag="xt")
                    nc.sync.dma_start(
                        out=xt, in_=xT_r[:, :, j * SBLK : (j + 1) * SBLK]
                    )

                    # QT: 4 feature blocks of 128
                    for f in range(FSH // P):
                        ps = pa_ps.tile([P, SBLK], F32, tag="pa")
                        for k in range(KSUB):
                            nc.tensor.matmul(
                                ps,
                                wqkv_sb[:, k, f * P : (f + 1) * P],
                                xt[:, k, :],
                                start=(k == 0),
                                stop=(k == KSUB - 1),
                            )
                        e = etpool.tile([P, SBLK], F32, tag="e")
                        nc.scalar.activation(
                            out=e, in_=ps, func=mybir.ActivationFunctionType.Exp
                        )
                        r = etpool.tile([P, SBLK], F32, tag="r")
                        nc.vector.tensor_scalar_max(r, ps, 0.0)
                        nc.vector.scalar_tensor_tensor(
                            out=qt_sb[:, f, j * SBLK : (j + 1) * SBLK],
                            in0=e,
                            scalar=1.0,
                            in1=r,
                            op0=mybir.AluOpType.min,
                            op1=mybir.AluOpType.add,
                        )

                    # K, V token-major per 128-token subtile.
                    # vst carries a ones column per head-pair slot so one
                    # matmul accumulates both KV and K_sum^T.
                    kst = stpool.tile([P, TSUB, FSH], BF16, tag="kst")
                    vst = stpool.tile([P, TSUB, NPAIR, P + 1], BF16, tag="vst")
                    nc.vector.memset(vst[:, :, :, P : P + 1], 1.0)
                    for t in range(TSUB):
                        psk = pa_ps.tile([P, FSH], F32, tag="pa")
                        psv = pa_ps.tile([P, FSH], F32, tag="pa")
                        for k in range(KSUB):
                            nc.tensor.matmul(
                                psk,
                                xt[:, k, t * P : (t + 1) * P],
                                wqkv_sb[:, k, FSH : 2 * FSH],
                                start=(k == 0),
                                stop=(k == KSUB - 1),
                            )
                            nc.tensor.matmul(
                                psv,
                                xt[:, k, t * P : (t + 1) * P],
                                wqkv_sb[:, k, 2 * FSH : 3 * FSH],
                                start=(k == 0),
                                stop=(k == KSUB - 1),
                            )
                        e = etpool.tile([P, SBLK], F32, tag="e")
                        nc.scalar.activation(
                            out=e, in_=psk, func=mybir.ActivationFunctionType.Exp
                        )
                        r = etpool.tile([P, SBLK], F32, tag="r")
                        nc.vector.tensor_scalar_max(r, psk, 0.0)
                        nc.vector.scalar_tensor_tensor(
                            out=kst[:, t, :],
                            in0=e,
                            scalar=1.0,
                            in1=r,
                            op0=mybir.AluOpType.min,
                            op1=mybir.AluOpType.add,
                        )

                        nc.scalar.copy(out=vst[:, t, :, 0:P], in_=psv)

                    # phase B: accumulate [KV | K_sum^T] into persistent psums
                    first = j == 0
                    last = j == NBLK - 1
                    for t in range(TSUB):
                        for p_ in range(NPAIR):
                            nc.tensor.matmul(
                                kvps[p_],
                                kst[:, t, p_ * P : (p_ + 1) * P],
                                vst[:, t, p_, :],
                                start=(first and t == 0),
                                stop=(last and t == TSUB - 1),
                            )

                for p_ in range(NPAIR):
                    nc.vector.memset(ksumpad_sb[p_], 0.0)
                    nc.vector.tensor_copy(
                        out=ksumpad_sb[p_][0:HD, 0:1],
                        in_=kvps[p_][0:HD, P : P + 1],
                    )
                    nc.vector.tensor_copy(
                        out=ksumpad_sb[p_][HD:P, 1:2],
                        in_=kvps[p_][HD:P, P : P + 1],
                    )
                # per-pair block-diagonal KV lhsT (bf16)
                for p_ in range(NPAIR):
                    nc.vector.memset(lhsT2_sb[p_], 0.0)
                    nc.vector.tensor_copy(
                        out=lhsT2_sb[p_][0:HD, 0:HD],
                        in_=kvps[p_][0:HD, 0:HD],
                    )
                    nc.vector.tensor_copy(
                        out=lhsT2_sb[p_][HD:P, HD:P],
                        in_=kvps[p_][HD:P, HD:P],
                    )

            # ---------------- phase C + D ----------------
            # software-pipelined: C1(j) runs ahead while C3/D(j-1) finish.
            # Normalizer rows are computed by separate M=32 zero-padded
            # matmuls so all 8 land on 32-aligned partitions of 2 PSUM banks
            # -> Ln/Exp run 4 lanes wide instead of 1.
            with (
                tc.tile_pool(name="pcps", bufs=2, space="PSUM") as pc_ps,
                tc.tile_pool(name="pnps", bufs=1, space="PSUM") as pn_ps,
                tc.tile_pool(name="prps", bufs=2, space="PSUM") as pr_ps,
                tc.tile_pool(name="pyps", bufs=2, space="PSUM") as py_ps,
                tc.tile_pool(name="cd", bufs=2) as cdpool,
                tc.tile_pool(name="ou", bufs=2) as oupool,
                tc.tile_pool(name="rc", bufs=3) as rcpool,
                tc.tile_pool(name="yout", bufs=2) as ypool,
            ):
                outus = {}
                rcps = {}
                pscs_d = {}
                outts = {}

                def c_psc(j, ps_):
                    if j not in outus:
                        outus[j] = oupool.tile(
                            [P, NPAIR, SBLK], F32, tag="outu", name="outu"
                        )
                        pscs_d[j] = []
                    outu = outus[j]
                    for p_ in ps_:
                        psc = pc_ps.tile([P, SBLK], F32, tag="pc", name="psc")
                        nc.tensor.matmul(
                            psc,
                            lhsT2_sb[p_],
                            qt_sb[:, p_, j * SBLK : (j + 1) * SBLK],
                            start=True,
                            stop=True,
                        )
                        nc.scalar.copy(out=outu[:, p_, :], in_=psc)
                        pscs_d[j].append(psc)

                def c_norm_ln(j):
                    psn = pn_ps.tile([P, SBLK], F32, tag="pn", name="psn")
                    for p_ in range(NPAIR):
                        nc.tensor.matmul(
                            psn[32 * p_ : 32 * p_ + 32, :],
                            ksumpad_sb[p_],
                            qt_sb[:, p_, j * SBLK : (j + 1) * SBLK],
                            start=True,
                            stop=True,
                            tile_position=(0, 32 * p_),
                        )
                    nrmt = rcpool.tile([P, SBLK], F32, tag="nt", name="nrmt")
                    nc.scalar.activation(
                        out=nrmt,
                        in_=psn,
                        func=mybir.ActivationFunctionType.Ln,
                        bias=eps_sb,
                    )
                    return nrmt

                def c_norm_exp(j, nrmt):
                    rcpt = rcpool.tile([P, SBLK], F32R, tag="rc", name="rcpt")
                    with nc.allow_low_precision(
                        reason="fp32r is 32-bit; fp32r matmul operand"
                    ):
                        nc.scalar.activation(
                            out=rcpt,
                            in_=nrmt,
                            func=mybir.ActivationFunctionType.Exp,
                            scale=-1.0,
                        )
                    rcps[j] = rcpt

                def c_norm_pair(j):
                    # blocks j, j+1 normalizers with Ln,Ln,Exp,Exp ordering:
                    # one ACT table switch pair per TWO blocks
                    n0 = c_norm_ln(j)
                    n1 = c_norm_ln(j + 1)
                    c_norm_exp(j, n0)
                    c_norm_exp(j + 1, n1)

                def c_apply(j):
                    outu = outus.pop(j)
                    rcpt = rcps.pop(j)
                    pscs_d.pop(j)
                    outt = cdpool.tile(
                        [P, FSH // P, SBLK], BF16, tag="outt", name="outt"
                    )
                    outts[j] = outt
                    for p_ in range(NPAIR):
                        rb = 32 * p_
                        psr = pr_ps.tile([P, SBLK], F32, tag="pr", name="psr")
                        nc.tensor.matmul(
                            psr,
                            ones2_fr[rb : rb + 2, :],
                            rcpt[rb : rb + 2, :],
                            start=True,
                            stop=True,
                            tile_position=(rb, 0),
                        )
                        nc.vector.tensor_tensor(
                            out=outt[:, p_, :],
                            in0=outu[:, p_, :],
                            in1=psr,
                            op=mybir.AluOpType.mult,
                        )

                def d_half(j, th):
                    outt = outts[j]
                    ysb = ypool.tile([P, 2, D], F32, tag="ysb", name="ysb")
                    for t2 in range(2):
                        t = th * 2 + t2
                        for nb in range(D // 512):
                            psy = py_ps.tile([P, 512], F32, tag="py", name="psy")
                            for fs in range(FSH // P):
                                nc.tensor.matmul(
                                    psy,
                                    outt[:, fs, t * P : (t + 1) * P],
                                    wo_sb[:, fs, nb * 512 : (nb + 1) * 512],
                                    start=(fs == 0),
                                    stop=(fs == FSH // P - 1),
                                )
                            nc.vector.tensor_copy(
                                out=ysb[:, t2, nb * 512 : (nb + 1) * 512], in_=psy
                            )
                    nc.sync.dma_start(out=y_r2[j, :, th], in_=ysb)
                    if th == TSUB // 2 - 1:
                        outts.pop(j)

                # interleaved emission: previous block's broadcast/apply and
                # Wo matmuls slot between this block's psc matmuls so the PE
                # is never program-order-stalled on ACT evictions recycling
                # the psc PSUM banks.
                for j in range(NBLK):
                    c_psc(j, [0, 1])
                    if j >= 1:
                        c_apply(j - 1)
                    c_psc(j, [2, 3])
                    if j >= 1:
                        d_half(j - 1, 0)
                    if j % 2 == 0:
                        c_norm_pair(j)
                    if j >= 1:
                        d_half(j - 1, 1)
                c_apply(NBLK - 1)
                d_half(NBLK - 1, 0)
                d_half(NBLK - 1, 1)

    nc.compile()
    return nc


def _prep_inputs(x, Wqkv, Wo):
    import ml_dtypes

    x = np.ascontiguousarray(x, dtype=np.float32)
    Wqkv = np.ascontiguousarray(Wqkv, dtype=np.float32)
    Wo = np.ascontiguousarray(Wo, dtype=np.float32)
    in_maps = []
    for b in range(B):
        xT = np.ascontiguousarray(x[b].T).astype(ml_dtypes.bfloat16)  # [D, S]
        for hh in range(2):
            cols = slice(hh * FSH, (hh + 1) * FSH)
            wq = Wqkv[:, 0 * D :][:, cols]
            wk = Wqkv[:, 1 * D :][:, cols]
            wv = Wqkv[:, 2 * D :][:, cols]
            wqkv_sh = np.ascontiguousarray(
                np.concatenate([wq, wk, wv], axis=1)
            ).astype(ml_dtypes.bfloat16)
            wo_sh = np.ascontiguousarray(Wo[hh * FSH : (hh + 1) * FSH, :]).astype(
                ml_dtypes.bfloat16
            )
            ones2 = np.zeros((128, 128), dtype=np.float32)
            for k in range(4):
                ones2[32 * k, 0:64] = 1.0
                ones2[32 * k + 1, 64:128] = 1.0
            in_maps.append(
                {"xT": xT, "wqkv": wqkv_sh, "wo": wo_sh, "ones2": ones2}
            )
    return in_maps


def kernel(x, Wqkv, Wo):
    global _NC_CACHE
    if _NC_CACHE is None:
        _NC_CACHE = build()
    nc = _NC_CACHE
    in_maps = _prep_inputs(x, Wqkv, Wo)
    res = run_bass_kernel_spmd(nc, in_maps, list(range(2 * B))).results
    y = np.empty((B, S, D), dtype=np.float32)
    for b in range(B):
        y[b] = res[2 * b]["y"] + res[2 * b + 1]["y"]
    return y



# revision 3
# speedup vs baseline: 1.0973x; 1.0973x over previous
"""Linear attention (B=4, S=4096, D=1024, H=16) on 8 TRN2 NeuronCores.

Sharding: core = (batch, head-half): each core handles one batch's 8 heads.
 - x is host-transposed to xT [D, S] per batch so both operand orientations
   of every matmul come out of the tensor engine with no on-device transpose.
 - Wqkv column-sharded per head-half; Wo row-sharded; host sums the two
   partial y's per batch (row-parallel unshard).

Two-phase dataflow (all matmuls bf16, fp32 PSUM accumulate):

phase 1 (per 512-token block): K,V projection token-major (lhsT=xT slice,
  rhs=Wk/Wv) -> elu+1(K) -> [KV | K_sum^T] PSUM accumulation per head-pair
  (vst carries a ones column so one matmul does both). Q is NOT computed
  here -- it is deferred to phase 2 so the PE has independent work to chew
  on across the KV -> attention transition (no pipeline bubble), and so x
  (kept resident in SBUF, 8MB bf16) is the only phase-1 input.
  Block 0 runs k-outer (4 simultaneous PSUM chains, one per 128-token
  subtile) so compute starts as soon as the first (wkv, x) DMA chunk lands
  instead of waiting for the full weight load.

phase 2 (per block, software-pipelined across j):
  QT [512f, 512s] feature-major (lhsT=Wq, rhs=xT slice) -> elu+1 -> bf16
  psc[128,s] = blockdiag(KV_h0, KV_h1)^T @ QT_pair: both heads of a pair
    in one matmul; ACT-evicted to outu
  norm: lhsT = [ksum_h0 replicated x64 | ksum_h1 replicated x64] so the
    matmul output IS the normalizer broadcast across all 128 partitions
    (no separate broadcast matmul); 1/x via the single-instruction DVE
    fast reciprocal (no Ln/Exp ACT ops, no activation-table switches)
  outT = outu * rcp (one DVE mult per pair, bf16)
  y[s,:] = outT^T @ Wo per 128-token subtile, fp32 out, DMAed per subtile
    (512KB chunks) to keep the drain tail short.
"""

import numpy as np

import concourse.bacc as bacc
import concourse.mybir as mybir
import concourse.tile as tile
from concourse.bass_utils import run_bass_kernel_spmd

F32 = mybir.dt.float32
BF16 = mybir.dt.bfloat16
ACT = mybir.ActivationFunctionType

P = 128
B, S, D = 4, 4096, 1024
H = 16
HD = 64

FSH = 512            # features per core for each of Q, K, V (8 heads)
KSUB = D // P        # 8 contraction subtiles
SBLK = 512           # tokens per block
NBLK = S // SBLK     # 8 blocks
TSUB = SBLK // P     # 4 token subtiles per block
NPAIR = 4            # head pairs per core

_NC_CACHE = None


def build():
    nc = bacc.Bacc(target_bir_lowering=False)
    xT = nc.dram_tensor("xT", [D, S], BF16, kind="ExternalInput")
    wqkv = nc.dram_tensor("wqkv", [D, 3 * FSH], BF16, kind="ExternalInput")
    wo = nc.dram_tensor("wo", [FSH, D], BF16, kind="ExternalInput")
    y = nc.dram_tensor("y", [S, D], F32, kind="ExternalOutput")

    xT_r = xT.rearrange("(ko p) s -> p ko s", p=P)        # [128, 8, 4096]
    wqkv_r = wqkv.rearrange("(ko p) f -> p ko f", p=P)    # [128, 8, 1536]
    wo_r = wo.rearrange("(fo p) n -> p fo n", p=P)        # [128, 4, 1024]
    y_rt = y.rearrange("(j t p) n -> j t p n", t=TSUB, p=P)  # [8,4,128,1024]

    with tile.TileContext(nc) as tc:
        import contextlib

        with contextlib.ExitStack() as ctx:
            wpool = ctx.enter_context(tc.tile_pool(name="wpool", bufs=1))

            # persistent SBUF
            xt_sb = wpool.tile([P, KSUB, S], BF16)          # all of x, 64KB/p
            wqkv_sb = wpool.tile([P, KSUB, 3 * FSH], BF16)  # [wq|wk|wv]
            wo_sb = wpool.tile([P, FSH // P, D], BF16)
            # per-pair block-diagonal [[KV_h0, 0], [0, KV_h1]] (128x128)
            lhsT2_sb = [
                wpool.tile([P, P], BF16, name=f"l2{p}") for p in range(NPAIR)
            ]
            # per-pair [ksum_h0 x64 | ksum_h1 x64] replicated along free dim:
            # norm matmul output comes out already broadcast per head-half
            ksumrep_sb = [
                wpool.tile([P, P], BF16, name=f"kr{p}") for p in range(NPAIR)
            ]

            # DMA order: block-0-critical chunks first (wk|wv + x block 0,
            # interleaved per contraction subtile), then remaining x, then
            # the phase-2-only weights (wq, wo).
            for k in range(KSUB):
                nc.sync.dma_start(
                    out=wqkv_sb[:, k, FSH : 3 * FSH],
                    in_=wqkv_r[:, k, FSH : 3 * FSH],
                )
                nc.sync.dma_start(
                    out=xt_sb[:, k, 0:SBLK], in_=xT_r[:, k, 0:SBLK]
                )
            for p_ in range(NPAIR):
                nc.vector.memset(lhsT2_sb[p_], 0.0)
                nc.vector.memset(ksumrep_sb[p_], 0.0)
            for j in range(1, NBLK):
                nc.sync.dma_start(
                    out=xt_sb[:, :, j * SBLK : (j + 1) * SBLK],
                    in_=xT_r[:, :, j * SBLK : (j + 1) * SBLK],
                )
            for k in range(KSUB):
                nc.sync.dma_start(
                    out=wqkv_sb[:, k, 0:FSH], in_=wqkv_r[:, k, 0:FSH]
                )
            nc.sync.dma_start(out=wo_sb, in_=wo_r)

            # ---------------- phase 1: K,V projection + KV accumulation ----
            with (
                tc.tile_pool(name="kvps", bufs=1, space="PSUM") as kvps_pool,
                tc.tile_pool(name="pa", bufs=4, space="PSUM") as pa_pool,
                tc.tile_pool(name="st", bufs=2) as stpool,
                tc.tile_pool(name="et", bufs=3) as etpool,
            ):
                kvps = [
                    kvps_pool.tile([P, P + 1], F32, tag=f"kv{p}", name=f"kv{p}")
                    for p in range(NPAIR)
                ]

                bq = []  # lagged [KV | K_sum] accumulation entries

                def emit_b(ent):
                    kst, vst, j, t = ent
                    first = j == 0 and t == 0
                    last = j == NBLK - 1 and t == TSUB - 1
                    for p_ in range(NPAIR):
                        nc.tensor.matmul(
                            kvps[p_],
                            kst[:, t, p_ * P : (p_ + 1) * P],
                            vst[:, t, p_, :],
                            start=first,
                            stop=last,
                        )

                def elu_k(ps, kst, t):
                    e = etpool.tile([P, SBLK], F32, tag="e")
                    nc.scalar.activation(out=e, in_=ps, func=ACT.Exp)
                    r = etpool.tile([P, SBLK], F32, tag="r")
                    nc.vector.tensor_scalar_max(r, ps, 0.0)
                    nc.vector.scalar_tensor_tensor(
                        out=kst[:, t, :],
                        in0=e,
                        scalar=1.0,
                        in1=r,
                        op0=mybir.AluOpType.min,
                        op1=mybir.AluOpType.add,
                    )

                # block 0: k-outer so PE work tracks DMA chunk arrival
                kst0 = stpool.tile([P, TSUB, FSH], BF16, tag="kst")
                vst0 = stpool.tile([P, TSUB, NPAIR, P + 1], BF16, tag="vst")
                nc.vector.memset(vst0[:, :, :, P : P + 1], 1.0)
                psks = [
                    pa_pool.tile([P, SBLK], F32, tag="pa", name=f"psk{t}")
                    for t in range(TSUB)
                ]
                for k in range(KSUB):
                    for t in range(TSUB):
                        nc.tensor.matmul(
                            psks[t],
                            xt_sb[:, k, t * P : (t + 1) * P],
                            wqkv_sb[:, k, FSH : 2 * FSH],
                            start=(k == 0),
                            stop=(k == KSUB - 1),
                        )
                for t in range(TSUB):
                    elu_k(psks[t], kst0, t)
                for t in range(TSUB):
                    psv = pa_pool.tile([P, SBLK], F32, tag="pa", name=f"psv{t}")
                    for k in range(KSUB):
                        nc.tensor.matmul(
                            psv,
                            xt_sb[:, k, t * P : (t + 1) * P],
                            wqkv_sb[:, k, 2 * FSH : 3 * FSH],
                            start=(k == 0),
                            stop=(k == KSUB - 1),
                        )
                    if t >= 1:
                        emit_b(bq.pop(0))
                    nc.scalar.copy(out=vst0[:, t, :, 0:P], in_=psv)
                    bq.append((kst0, vst0, 0, t))

                # blocks 1..7: token-subtile-outer, B lagged one step
                for j in range(1, NBLK):
                    kst = stpool.tile([P, TSUB, FSH], BF16, tag="kst")
                    vst = stpool.tile([P, TSUB, NPAIR, P + 1], BF16, tag="vst")
                    nc.vector.memset(vst[:, :, :, P : P + 1], 1.0)
                    for t in range(TSUB):
                        tok = j * SBLK + t * P
                        psk = pa_pool.tile([P, SBLK], F32, tag="pa")
                        psv = pa_pool.tile([P, SBLK], F32, tag="pa")
                        for k in range(KSUB):
                            nc.tensor.matmul(
                                psk,
                                xt_sb[:, k, tok : tok + P],
                                wqkv_sb[:, k, FSH : 2 * FSH],
                                start=(k == 0),
                                stop=(k == KSUB - 1),
                            )
                            nc.tensor.matmul(
                                psv,
                                xt_sb[:, k, tok : tok + P],
                                wqkv_sb[:, k, 2 * FSH : 3 * FSH],
                                start=(k == 0),
                                stop=(k == KSUB - 1),
                            )
                        emit_b(bq.pop(0))
                        elu_k(psk, kst, t)
                        nc.scalar.copy(out=vst[:, t, :, 0:P], in_=psv)
                        bq.append((kst, vst, j, t))
                emit_b(bq.pop(0))

                # extraction: blockdiag KV + replicated K_sum (zeros preset)
                for p_ in range(NPAIR):
                    nc.vector.tensor_copy(
                        out=lhsT2_sb[p_][0:HD, 0:HD], in_=kvps[p_][0:HD, 0:HD]
                    )
                    nc.vector.tensor_copy(
                        out=lhsT2_sb[p_][HD:P, HD:P], in_=kvps[p_][HD:P, HD:P]
                    )
                    nc.vector.tensor_copy(
                        out=ksumrep_sb[p_][0:HD, 0:HD],
                        in_=kvps[p_][0:HD, P : P + 1].to_broadcast((HD, HD)),
                    )
                    nc.vector.tensor_copy(
                        out=ksumrep_sb[p_][HD:P, HD:P],
                        in_=kvps[p_][HD:P, P : P + 1].to_broadcast((HD, HD)),
                    )

            # ---------------- phase 2: Q projection + attention + Wo -------
            with (
                tc.tile_pool(name="mm512", bufs=3, space="PSUM") as mmps,
                tc.tile_pool(name="pc", bufs=3, space="PSUM") as pcps,
                tc.tile_pool(name="pnb", bufs=2, space="PSUM") as pnps,
                tc.tile_pool(name="qt", bufs=2) as qtpool,
                tc.tile_pool(name="et2", bufs=3) as etpool2,
                tc.tile_pool(name="ou", bufs=2) as oupool,
                tc.tile_pool(name="rc", bufs=4) as rcpool,
                tc.tile_pool(name="ot", bufs=2) as otpool,
                tc.tile_pool(name="ys", bufs=2) as ypool,
            ):
                qts = {}
                outus = {}
                rcbs = {}
                outts = {}

                def qt_half(j, fh):
                    if j not in qts:
                        qts[j] = qtpool.tile(
                            [P, NPAIR, SBLK], BF16, tag="qt", name=f"qt{j}"
                        )
                    for f in (2 * fh, 2 * fh + 1):
                        ps = mmps.tile([P, SBLK], F32, tag="mm")
                        for k in range(KSUB):
                            nc.tensor.matmul(
                                ps,
                                wqkv_sb[:, k, f * P : (f + 1) * P],
                                xt_sb[:, k, j * SBLK : (j + 1) * SBLK],
                                start=(k == 0),
                                stop=(k == KSUB - 1),
                            )
                        e = etpool2.tile([P, SBLK], F32, tag="e2")
                        nc.scalar.activation(out=e, in_=ps, func=ACT.Exp)
                        r = etpool2.tile([P, SBLK], F32, tag="r2")
                        nc.scalar.activation(out=r, in_=ps, func=ACT.Relu)
                        nc.vector.scalar_tensor_tensor(
                            out=qts[j][:, f, :],
                            in0=e,
                            scalar=1.0,
                            in1=r,
                            op0=mybir.AluOpType.min,
                            op1=mybir.AluOpType.add,
                        )

                def psc_section(j):
                    # per pair: attention matmul (ACT-evicted) + broadcast
                    # normalizer matmul (DVE fast reciprocal, stays in SBUF)
                    qtj = qts.pop(j)
                    outu = oupool.tile([P, NPAIR, SBLK], F32, tag="outu")
                    outus[j] = outu
                    rcbs[j] = []
                    for p_ in range(NPAIR):
                        psc = pcps.tile([P, SBLK], F32, tag="pc")
                        nc.tensor.matmul(
                            psc,
                            lhsT2_sb[p_],
                            qtj[:, p_, :],
                            start=True,
                            stop=True,
                        )
                        nc.scalar.copy(out=outu[:, p_, :], in_=psc)
                        psn = pnps.tile([P, SBLK], F32, tag="pn")
                        nc.tensor.matmul(
                            psn,
                            ksumrep_sb[p_],
                            qtj[:, p_, :],
                            start=True,
                            stop=True,
                        )
                        rcb = rcpool.tile([P, SBLK], F32, tag="rcb")
                        nc.vector.reciprocal_approx_fast(out=rcb[:], in_=psn[:])
                        rcbs[j].append(rcb)

                def mults(j):
                    outt = otpool.tile([P, NPAIR, SBLK], BF16, tag="outt")
                    outts[j] = outt
                    outu = outus.pop(j)
                    rcs = rcbs.pop(j)
                    for p_ in range(NPAIR):
                        nc.vector.tensor_tensor(
                            out=outt[:, p_, :],
                            in0=outu[:, p_, :],
                            in1=rcs[p_],
                            op=mybir.AluOpType.mult,
                        )

                def d_block(j):
                    outt = outts.pop(j)
                    for t in range(TSUB):
                        ysb = ypool.tile([P, D], F32, tag="ysb")
                        psy0 = mmps.tile([P, 512], F32, tag="mm")
                        psy1 = mmps.tile([P, 512], F32, tag="mm")
                        for fs in range(FSH // P):
                            nc.tensor.matmul(
                                psy0,
                                outt[:, fs, t * P : (t + 1) * P],
                                wo_sb[:, fs, 0:512],
                                start=(fs == 0),
                                stop=(fs == FSH // P - 1),
                            )
                            nc.tensor.matmul(
                                psy1,
                                outt[:, fs, t * P : (t + 1) * P],
                                wo_sb[:, fs, 512:1024],
                                start=(fs == 0),
                                stop=(fs == FSH // P - 1),
                            )
                        nc.scalar.copy(out=ysb[:, 0:512], in_=psy0)
                        nc.scalar.copy(out=ysb[:, 512:1024], in_=psy1)
                        nc.sync.dma_start(out=y_rt[j, t], in_=ysb)

                # steady-state emission: block j's Q projection brackets
                # block j-1's attention chain so the PE never waits on the
                # ACT/DVE eviction+reciprocal+apply latency.
                for j in range(NBLK):
                    if j >= 1:
                        psc_section(j - 1)
                    qt_half(j, 0)
                    if j >= 1:
                        mults(j - 1)
                    qt_half(j, 1)
                    if j >= 1:
                        d_block(j - 1)
                psc_section(NBLK - 1)
                mults(NBLK - 1)
                d_block(NBLK - 1)

    nc.compile()
    return nc


def _prep_inputs(x, Wqkv, Wo):
    import ml_dtypes

    x = np.ascontiguousarray(x, dtype=np.float32)
    Wqkv = np.ascontiguousarray(Wqkv, dtype=np.float32)
    Wo = np.ascontiguousarray(Wo, dtype=np.float32)
    in_maps = []
    for b in range(B):
        xT = np.ascontiguousarray(x[b].T).astype(ml_dtypes.bfloat16)  # [D, S]
        for hh in range(2):
            cols = slice(hh * FSH, (hh + 1) * FSH)
            wq = Wqkv[:, 0 * D :][:, cols]
            wk = Wqkv[:, 1 * D :][:, cols]
            wv = Wqkv[:, 2 * D :][:, cols]
            wqkv_sh = np.ascontiguousarray(
                np.concatenate([wq, wk, wv], axis=1)
            ).astype(ml_dtypes.bfloat16)
            wo_sh = np.ascontiguousarray(Wo[hh * FSH : (hh + 1) * FSH, :]).astype(
                ml_dtypes.bfloat16
            )
            in_maps.append({"xT": xT, "wqkv": wqkv_sh, "wo": wo_sh})
    return in_maps


def kernel(x, Wqkv, Wo):
    global _NC_CACHE
    if _NC_CACHE is None:
        _NC_CACHE = build()
    nc = _NC_CACHE
    in_maps = _prep_inputs(x, Wqkv, Wo)
    res = run_bass_kernel_spmd(nc, in_maps, list(range(2 * B))).results
    y = np.empty((B, S, D), dtype=np.float32)
    for b in range(B):
        y[b] = res[2 * b]["y"] + res[2 * b + 1]["y"]
    return y


# revision 10
# speedup vs baseline: 1.1258x; 1.0259x over previous
"""Linear attention (B=4, S=4096, D=1024, H=16) on 8 TRN2 NeuronCores.

Sharding: core = (batch, head-half): each core handles one batch's 8 heads.
 - x is host-transposed to xT [D, S] per batch so both operand orientations
   of every matmul come out of the tensor engine with no on-device transpose.
 - Wqkv column-sharded per head-half; Wo row-sharded; host sums the two
   partial y's per batch (row-parallel unshard).

Two-phase dataflow (all matmuls bf16, fp32 PSUM accumulate):

phase 1 (per 512-token block): K,V projection token-major (lhsT=xT slice,
  rhs=Wk/Wv) -> elu+1(K) -> [KV | K_sum^T] PSUM accumulation per head-pair
  (vst carries a ones column so one matmul does both). Q is NOT computed
  here -- it is deferred to phase 2 so the PE has independent work to chew
  on across the KV -> attention transition (no pipeline bubble), and so x
  (kept resident in SBUF, 8MB bf16) is the only phase-1 input.
  Block 0 runs k-outer (4 simultaneous PSUM chains, one per 128-token
  subtile) so compute starts as soon as the first (wkv, x) DMA chunk lands
  instead of waiting for the full weight load.

phase 2 (per block, software-pipelined across j):
  QT [512f, 512s] feature-major (lhsT=Wq, rhs=xT slice) -> elu+1 -> bf16
  psc[128,s] = blockdiag(KV_h0, KV_h1)^T @ QT_pair: both heads of a pair
    in one matmul; ACT-evicted to outu
  norm: lhsT = [ksum_h0 replicated x64 | ksum_h1 replicated x64] so the
    matmul output IS the normalizer broadcast across all 128 partitions
    (no separate broadcast matmul); 1/x via the single-instruction DVE
    fast reciprocal (no Ln/Exp ACT ops, no activation-table switches)
  outT = outu * rcp (one DVE mult per pair, bf16)
  y[s,:] = outT^T @ Wo per 128-token subtile, fp32 out, DMAed per subtile
    (512KB chunks) to keep the drain tail short.
"""

import numpy as np

import concourse.bacc as bacc
import concourse.mybir as mybir
import concourse.tile as tile
from concourse.bass_utils import run_bass_kernel_spmd

F32 = mybir.dt.float32
BF16 = mybir.dt.bfloat16
ACT = mybir.ActivationFunctionType

P = 128
B, S, D = 4, 4096, 1024
H = 16
HD = 64

FSH = 512            # features per core for each of Q, K, V (8 heads)
KSUB = D // P        # 8 contraction subtiles
SBLK = 512           # tokens per block
NBLK = S // SBLK     # 8 blocks
TSUB = SBLK // P     # 4 token subtiles per block
NPAIR = 4            # head pairs per core

_NC_CACHE = None


def build():
    nc = bacc.Bacc(target_bir_lowering=False)
    xT = nc.dram_tensor("xT", [D, S], BF16, kind="ExternalInput")
    wqkv = nc.dram_tensor("wqkv", [D, 3 * FSH], BF16, kind="ExternalInput")
    wo = nc.dram_tensor("wo", [FSH, D], BF16, kind="ExternalInput")
    y = nc.dram_tensor("y", [S, D], F32, kind="ExternalOutput")

    xT_r = xT.rearrange("(ko p) s -> p ko s", p=P)        # [128, 8, 4096]
    wqkv_r = wqkv.rearrange("(ko p) f -> p ko f", p=P)    # [128, 8, 1536]
    wo_r = wo.rearrange("(fo p) n -> p fo n", p=P)        # [128, 4, 1024]
    y_rt = y.rearrange(
        "(j t p) (nh n) -> j t nh p n", t=TSUB, p=P, nh=2
    )  # [8,4,2,128,512]

    with tile.TileContext(nc) as tc:
        import contextlib

        with contextlib.ExitStack() as ctx:
            wpool = ctx.enter_context(tc.tile_pool(name="wpool", bufs=1))

            # persistent SBUF
            xt_sb = wpool.tile([P, KSUB, S], BF16)          # all of x, 64KB/p
            wqkv_sb = wpool.tile([P, KSUB, 3 * FSH], BF16)  # [wq|wk|wv]
            wo_sb = wpool.tile([P, FSH // P, D], BF16)
            # per-pair block-diagonal [[KV_h0, 0], [0, KV_h1]] (128x128)
            lhsT2_sb = [
                wpool.tile([P, P], BF16, name=f"l2{p}") for p in range(NPAIR)
            ]
            # per-pair [ksum_h0 x64 | ksum_h1 x64] replicated along free dim:
            # norm matmul output comes out already broadcast per head-half
            ksumrep_sb = [
                wpool.tile([P, P], BF16, name=f"kr{p}") for p in range(NPAIR)
            ]

            # DMA order: block-0 V-sweep-critical chunks first (x block 0 +
            # wv, interleaved per contraction subtile), then wk (K-sweep),
            # then remaining x, then the phase-2-only weights (wq, wo).
            for k in range(KSUB):
                nc.sync.dma_start(
                    out=xt_sb[:, k, 0:SBLK], in_=xT_r[:, k, 0:SBLK]
                )
                nc.sync.dma_start(
                    out=wqkv_sb[:, k, 2 * FSH : 3 * FSH],
                    in_=wqkv_r[:, k, 2 * FSH : 3 * FSH],
                )
            for k in range(KSUB):
                nc.sync.dma_start(
                    out=wqkv_sb[:, k, FSH : 2 * FSH],
                    in_=wqkv_r[:, k, FSH : 2 * FSH],
                )
            for p_ in range(NPAIR):
                nc.vector.memset(lhsT2_sb[p_], 0.0)
                nc.vector.memset(ksumrep_sb[p_], 0.0)
            for j in range(1, NBLK):
                nc.sync.dma_start(
                    out=xt_sb[:, :, j * SBLK : (j + 1) * SBLK],
                    in_=xT_r[:, :, j * SBLK : (j + 1) * SBLK],
                )
            for k in range(KSUB):
                nc.sync.dma_start(
                    out=wqkv_sb[:, k, 0:FSH], in_=wqkv_r[:, k, 0:FSH]
                )
            nc.sync.dma_start(out=wo_sb, in_=wo_r)

            # SBUF pools shared across both phases
            etpool = ctx.enter_context(tc.tile_pool(name="et", bufs=3))
            qtpool = ctx.enter_context(tc.tile_pool(name="qt", bufs=2))
            qts = {}

            def qt_elu(ps, j, f):
                # elu(x)+1 = min(exp(x),1) + relu(x); Exp/Relu on ACT, the
                # combine on DVE
                e = etpool.tile([P, SBLK], F32, tag="e")
                nc.scalar.activation(out=e, in_=ps, func=ACT.Exp)
                r = etpool.tile([P, SBLK], F32, tag="r")
                nc.scalar.activation(out=r, in_=ps, func=ACT.Relu)
                nc.vector.scalar_tensor_tensor(
                    out=qts[j][:, f, :],
                    in0=e,
                    scalar=1.0,
                    in1=r,
                    op0=mybir.AluOpType.min,
                    op1=mybir.AluOpType.add,
                )

            # ---------------- phase 1: K,V projection + KV accumulation ----
            with (
                tc.tile_pool(name="kvps", bufs=1, space="PSUM") as kvps_pool,
                tc.tile_pool(name="pa", bufs=4, space="PSUM") as pa_pool,
                tc.tile_pool(name="st", bufs=2) as stpool,
            ):
                kvps = [
                    kvps_pool.tile([P, P + 1], F32, tag=f"kv{p}", name=f"kv{p}")
                    for p in range(NPAIR)
                ]

                bq = []  # lagged [KV | K_sum] accumulation entries

                def emit_b(ent):
                    kst, vst, j, t = ent
                    first = j == 0 and t == 0
                    last = j == NBLK - 1 and t == TSUB - 1
                    for p_ in range(NPAIR):
                        nc.tensor.matmul(
                            kvps[p_],
                            kst[:, t, p_ * P : (p_ + 1) * P],
                            vst[:, t, p_, :],
                            start=first,
                            stop=last,
                        )

                def elu_k(ps, kst, t):
                    e = etpool.tile([P, SBLK], F32, tag="e")
                    nc.scalar.activation(out=e, in_=ps, func=ACT.Exp)
                    r = etpool.tile([P, SBLK], F32, tag="r")
                    nc.vector.tensor_scalar_max(r, ps, 0.0)
                    nc.vector.scalar_tensor_tensor(
                        out=kst[:, t, :],
                        in0=e,
                        scalar=1.0,
                        in1=r,
                        op0=mybir.AluOpType.min,
                        op1=mybir.AluOpType.add,
                    )

                # block 0: V-sweep k-outer so PE work tracks DMA chunk
                # arrival (V needs no activation, so the K sweep that
                # follows runs at PE speed with elu pipelined per subtile)
                kst0 = stpool.tile([P, TSUB, FSH], BF16, tag="kst")
                vst0 = stpool.tile([P, TSUB, NPAIR, P + 1], BF16, tag="vst")
                nc.vector.memset(vst0[:, :, :, P : P + 1], 1.0)
                psvs = [
                    pa_pool.tile([P, SBLK], F32, tag="pa", name=f"psv{t}")
                    for t in range(TSUB)
                ]
                for k in range(KSUB):
                    for t in range(TSUB):
                        nc.tensor.matmul(
                            psvs[t],
                            xt_sb[:, k, t * P : (t + 1) * P],
                            wqkv_sb[:, k, 2 * FSH : 3 * FSH],
                            start=(k == 0),
                            stop=(k == KSUB - 1),
                        )
                for t in range(TSUB):
                    nc.scalar.copy(out=vst0[:, t, :, 0:P], in_=psvs[t])
                for t in range(TSUB):
                    psk = pa_pool.tile([P, SBLK], F32, tag="pa", name=f"psk{t}")
                    for k in range(KSUB):
                        nc.tensor.matmul(
                            psk,
                            xt_sb[:, k, t * P : (t + 1) * P],
                            wqkv_sb[:, k, FSH : 2 * FSH],
                            start=(k == 0),
                            stop=(k == KSUB - 1),
                        )
                    if t >= 1:
                        emit_b(bq.pop(0))
                    elu_k(psk, kst0, t)
                    bq.append((kst0, vst0, 0, t))

                # blocks 1..7: token-subtile-outer, B lagged one step
                for j in range(1, NBLK):
                    kst = stpool.tile([P, TSUB, FSH], BF16, tag="kst")
                    vst = stpool.tile([P, TSUB, NPAIR, P + 1], BF16, tag="vst")
                    nc.vector.memset(vst[:, :, :, P : P + 1], 1.0)
                    for t in range(TSUB):
                        tok = j * SBLK + t * P
                        psk = pa_pool.tile([P, SBLK], F32, tag="pa")
                        psv = pa_pool.tile([P, SBLK], F32, tag="pa")
                        for k in range(KSUB):
                            nc.tensor.matmul(
                                psk,
                                xt_sb[:, k, tok : tok + P],
                                wqkv_sb[:, k, FSH : 2 * FSH],
                                start=(k == 0),
                                stop=(k == KSUB - 1),
                            )
                            nc.tensor.matmul(
                                psv,
                                xt_sb[:, k, tok : tok + P],
                                wqkv_sb[:, k, 2 * FSH : 3 * FSH],
                                start=(k == 0),
                                stop=(k == KSUB - 1),
                            )
                        emit_b(bq.pop(0))
                        elu_k(psk, kst, t)
                        nc.scalar.copy(out=vst[:, t, :, 0:P], in_=psv)
                        bq.append((kst, vst, j, t))
                # block 0's Q projection runs here, inside the phase-1 PSUM
                # pools: it has no dependency on the KV state, so it keeps
                # the PE busy across the phase boundary (the trailing elu,
                # the KV extraction, and the phase-2 pool handover all hide
                # under its 32 matmuls)
                qts[0] = qtpool.tile([P, NPAIR, SBLK], BF16, tag="qt", name="qt0")
                for f in range(FSH // P):
                    psq = pa_pool.tile([P, SBLK], F32, tag="pa")
                    for k in range(KSUB):
                        nc.tensor.matmul(
                            psq,
                            wqkv_sb[:, k, f * P : (f + 1) * P],
                            xt_sb[:, k, 0:SBLK],
                            start=(k == 0),
                            stop=(k == KSUB - 1),
                        )
                    if f == 0:
                        emit_b(bq.pop(0))
                    qt_elu(psq, 0, f)

                # extraction: blockdiag KV + replicated K_sum (zeros preset)
                for p_ in range(NPAIR):
                    nc.vector.tensor_copy(
                        out=lhsT2_sb[p_][0:HD, 0:HD], in_=kvps[p_][0:HD, 0:HD]
                    )
                    nc.vector.tensor_copy(
                        out=lhsT2_sb[p_][HD:P, HD:P], in_=kvps[p_][HD:P, HD:P]
                    )
                    nc.vector.tensor_copy(
                        out=ksumrep_sb[p_][0:HD, 0:HD],
                        in_=kvps[p_][0:HD, P : P + 1].to_broadcast((HD, HD)),
                    )
                    nc.vector.tensor_copy(
                        out=ksumrep_sb[p_][HD:P, HD:P],
                        in_=kvps[p_][HD:P, P : P + 1].to_broadcast((HD, HD)),
                    )

            # ---------------- phase 2: Q projection + attention + Wo -------
            with (
                tc.tile_pool(name="mm512", bufs=3, space="PSUM") as mmps,
                tc.tile_pool(name="pc", bufs=3, space="PSUM") as pcps,
                tc.tile_pool(name="pnb", bufs=2, space="PSUM") as pnps,
                tc.tile_pool(name="ou", bufs=2) as oupool,
                tc.tile_pool(name="rc", bufs=4) as rcpool,
                tc.tile_pool(name="ot", bufs=2) as otpool,
                tc.tile_pool(name="ys", bufs=2) as ypool,
            ):
                outus = {}
                rcbs = {}
                outts = {}

                def qt_half(j, fh):
                    if j not in qts:
                        qts[j] = qtpool.tile(
                            [P, NPAIR, SBLK], BF16, tag="qt", name=f"qt{j}"
                        )
                    for f in (2 * fh, 2 * fh + 1):
                        ps = mmps.tile([P, SBLK], F32, tag="mm")
                        for k in range(KSUB):
                            nc.tensor.matmul(
                                ps,
                                wqkv_sb[:, k, f * P : (f + 1) * P],
                                xt_sb[:, k, j * SBLK : (j + 1) * SBLK],
                                start=(k == 0),
                                stop=(k == KSUB - 1),
                            )
                        qt_elu(ps, j, f)

                def psc_section(j):
                    # per pair: attention matmul (ACT-evicted) + broadcast
                    # normalizer matmul (DVE fast reciprocal, stays in SBUF)
                    qtj = qts.pop(j)
                    outu = oupool.tile([P, NPAIR, SBLK], F32, tag="outu")
                    outus[j] = outu
                    rcbs[j] = []
                    for p_ in range(NPAIR):
                        psc = pcps.tile([P, SBLK], F32, tag="pc")
                        nc.tensor.matmul(
                            psc,
                            lhsT2_sb[p_],
                            qtj[:, p_, :],
                            start=True,
                            stop=True,
                        )
                        nc.scalar.copy(out=outu[:, p_, :], in_=psc)
                        psn = pnps.tile([P, SBLK], F32, tag="pn")
                        nc.tensor.matmul(
                            psn,
                            ksumrep_sb[p_],
                            qtj[:, p_, :],
                            start=True,
                            stop=True,
                        )
                        rcb = rcpool.tile([P, SBLK], F32, tag="rcb")
                        nc.vector.reciprocal_approx_fast(out=rcb[:], in_=psn[:])
                        rcbs[j].append(rcb)

                def mults(j):
                    outt = otpool.tile([P, NPAIR, SBLK], BF16, tag="outt")
                    outts[j] = outt
                    outu = outus.pop(j)
                    rcs = rcbs.pop(j)
                    for p_ in range(NPAIR):
                        nc.vector.tensor_tensor(
                            out=outt[:, p_, :],
                            in0=outu[:, p_, :],
                            in1=rcs[p_],
                            op=mybir.AluOpType.mult,
                        )

                def d_t(j, outt, t):
                    ysb = ypool.tile([P, D], F32, tag="ysb", name="ysb")
                    psy0 = mmps.tile([P, 512], F32, tag="mm", name="psy0")
                    psy1 = mmps.tile([P, 512], F32, tag="mm", name="psy1")
                    for fs in range(FSH // P):
                        nc.tensor.matmul(
                            psy0,
                            outt[:, fs, t * P : (t + 1) * P],
                            wo_sb[:, fs, 0:512],
                            start=(fs == 0),
                            stop=(fs == FSH // P - 1),
                        )
                        nc.tensor.matmul(
                            psy1,
                            outt[:, fs, t * P : (t + 1) * P],
                            wo_sb[:, fs, 512:1024],
                            start=(fs == 0),
                            stop=(fs == FSH // P - 1),
                        )
                    nc.scalar.copy(out=ysb[:, 0:512], in_=psy0)
                    nc.sync.dma_start(out=y_rt[j, t, 0], in_=ysb[:, 0:512])
                    nc.scalar.copy(out=ysb[:, 512:1024], in_=psy1)
                    nc.sync.dma_start(out=y_rt[j, t, 1], in_=ysb[:, 512:1024])

                def d_block(j):
                    outt = outts.pop(j)
                    for t in range(TSUB):
                        d_t(j, outt, t)

                def finale(j):
                    # drain block: apply-multiplies split per token subtile
                    # so each D chain starts as soon as its slice is scaled
                    outt = otpool.tile([P, NPAIR, SBLK], BF16, tag="outt")
                    outu = outus.pop(j)
                    rcs = rcbs.pop(j)
                    for t in range(TSUB):
                        sl = slice(t * P, (t + 1) * P)
                        for p_ in range(NPAIR):
                            nc.vector.tensor_tensor(
                                out=outt[:, p_, sl],
                                in0=outu[:, p_, sl],
                                in1=rcs[p_][:, sl],
                                op=mybir.AluOpType.mult,
                            )
                        d_t(j, outt, t)

                # steady-state emission: block j's Q projection brackets
                # block j-1's attention chain so the PE never waits on the
                # ACT/DVE eviction+reciprocal+apply latency.
                for j in range(1, NBLK):
                    psc_section(j - 1)
                    qt_half(j, 0)
                    mults(j - 1)
                    qt_half(j, 1)
                    d_block(j - 1)
                psc_section(NBLK - 1)
                finale(NBLK - 1)

    nc.compile()
    return nc


def _prep_inputs(x, Wqkv, Wo):
    import ml_dtypes

    x = np.ascontiguousarray(x, dtype=np.float32)
    Wqkv = np.ascontiguousarray(Wqkv, dtype=np.float32)
    Wo = np.ascontiguousarray(Wo, dtype=np.float32)
    in_maps = []
    for b in range(B):
        xT = np.ascontiguousarray(x[b].T).astype(ml_dtypes.bfloat16)  # [D, S]
        for hh in range(2):
            cols = slice(hh * FSH, (hh + 1) * FSH)
            wq = Wqkv[:, 0 * D :][:, cols]
            wk = Wqkv[:, 1 * D :][:, cols]
            wv = Wqkv[:, 2 * D :][:, cols]
            wqkv_sh = np.ascontiguousarray(
                np.concatenate([wq, wk, wv], axis=1)
            ).astype(ml_dtypes.bfloat16)
            wo_sh = np.ascontiguousarray(Wo[hh * FSH : (hh + 1) * FSH, :]).astype(
                ml_dtypes.bfloat16
            )
            in_maps.append({"xT": xT, "wqkv": wqkv_sh, "wo": wo_sh})
    return in_maps


def kernel(x, Wqkv, Wo):
    global _NC_CACHE
    if _NC_CACHE is None:
        _NC_CACHE = build()
    nc = _NC_CACHE
    in_maps = _prep_inputs(x, Wqkv, Wo)
    res = run_bass_kernel_spmd(nc, in_maps, list(range(2 * B))).results
    y = np.empty((B, S, D), dtype=np.float32)
    for b in range(B):
        y[b] = res[2 * b]["y"] + res[2 * b + 1]["y"]
    return y


# revision 12
# speedup vs baseline: 1.1307x; 1.0044x over previous
"""Linear attention (B=4, S=4096, D=1024, H=16) on 8 TRN2 NeuronCores.

Sharding: core = (batch, head-half): each core handles one batch's 8 heads.
 - x is host-transposed to xT [D, S] per batch so both operand orientations
   of every matmul come out of the tensor engine with no on-device transpose.
 - Wqkv column-sharded per head-half; Wo row-sharded; host sums the two
   partial y's per batch (row-parallel unshard).

Two-phase dataflow (all matmuls bf16, fp32 PSUM accumulate):

phase 1 (per 512-token block): K,V projection token-major (lhsT=xT slice,
  rhs=Wk/Wv) -> elu+1(K) -> [KV | K_sum^T] PSUM accumulation per head-pair
  (vst carries a ones column so one matmul does both). Q is NOT computed
  here -- it is deferred to phase 2 so the PE has independent work to chew
  on across the KV -> attention transition (no pipeline bubble), and so x
  (kept resident in SBUF, 8MB bf16) is the only phase-1 input.
  Block 0 runs k-outer (4 simultaneous PSUM chains, one per 128-token
  subtile) so compute starts as soon as the first (wkv, x) DMA chunk lands
  instead of waiting for the full weight load.

phase 2 (per block, software-pipelined across j):
  QT [512f, 512s] feature-major (lhsT=Wq, rhs=xT slice) -> elu+1 -> bf16
  psc[128,s] = blockdiag(KV_h0, KV_h1)^T @ QT_pair: both heads of a pair
    in one matmul; ACT-evicted to outu
  norm: lhsT = [ksum_h0 replicated x64 | ksum_h1 replicated x64] so the
    matmul output IS the normalizer broadcast across all 128 partitions
    (no separate broadcast matmul); 1/x via the single-instruction DVE
    fast reciprocal (no Ln/Exp ACT ops, no activation-table switches)
  outT = outu * rcp (one DVE mult per pair, bf16)
  y[s,:] = outT^T @ Wo per 128-token subtile, fp32 out, DMAed per subtile
    (512KB chunks) to keep the drain tail short.
"""

import numpy as np

import concourse.bacc as bacc
import concourse.mybir as mybir
import concourse.tile as tile
from concourse.bass_utils import run_bass_kernel_spmd

F32 = mybir.dt.float32
BF16 = mybir.dt.bfloat16
ACT = mybir.ActivationFunctionType

P = 128
B, S, D = 4, 4096, 1024
H = 16
HD = 64

FSH = 512            # features per core for each of Q, K, V (8 heads)
KSUB = D // P        # 8 contraction subtiles
SBLK = 512           # tokens per block
NBLK = S // SBLK     # 8 blocks
TSUB = SBLK // P     # 4 token subtiles per block
NPAIR = 4            # head pairs per core

_NC_CACHE = None


def build():
    nc = bacc.Bacc(target_bir_lowering=False)
    xT = nc.dram_tensor("xT", [D, S], BF16, kind="ExternalInput")
    wqkv = nc.dram_tensor("wqkv", [D, 3 * FSH], BF16, kind="ExternalInput")
    wo = nc.dram_tensor("wo", [FSH, D], BF16, kind="ExternalInput")
    y = nc.dram_tensor("y", [S, D], F32, kind="ExternalOutput")

    xT_r = xT.rearrange("(ko p) s -> p ko s", p=P)        # [128, 8, 4096]
    wqkv_r = wqkv.rearrange("(ko p) f -> p ko f", p=P)    # [128, 8, 1536]
    wo_r = wo.rearrange("(fo p) n -> p fo n", p=P)        # [128, 4, 1024]
    y_rt = y.rearrange(
        "(j t p) (nh n) -> j t nh p n", t=TSUB, p=P, nh=2
    )  # [8,4,2,128,512]

    with tile.TileContext(nc) as tc:
        import contextlib

        with contextlib.ExitStack() as ctx:
            wpool = ctx.enter_context(tc.tile_pool(name="wpool", bufs=1))

            # persistent SBUF
            xt_sb = wpool.tile([P, KSUB, S], BF16)          # all of x, 64KB/p
            wqkv_sb = wpool.tile([P, KSUB, 3 * FSH], BF16)  # [wq|wk|wv]
            wo_sb = wpool.tile([P, FSH // P, D], BF16)
            # per-pair block-diagonal [[KV_h0, 0], [0, KV_h1]] (128x128)
            lhsT2_sb = [
                wpool.tile([P, P], BF16, name=f"l2{p}") for p in range(NPAIR)
            ]
            # per-pair [ksum_h0 x64 | ksum_h1 x64] replicated along free dim:
            # norm matmul output comes out already broadcast per head-half
            ksumrep_sb = [
                wpool.tile([P, P], BF16, name=f"kr{p}") for p in range(NPAIR)
            ]

            # DMA order: block-0 V-sweep-critical chunks first (x block 0 +
            # wv, interleaved per contraction subtile), then wk (K-sweep),
            # then remaining x, then the phase-2-only weights (wq, wo).
            for k in range(KSUB):
                nc.sync.dma_start(
                    out=xt_sb[:, k, 0:SBLK], in_=xT_r[:, k, 0:SBLK]
                )
                nc.sync.dma_start(
                    out=wqkv_sb[:, k, 2 * FSH : 3 * FSH],
                    in_=wqkv_r[:, k, 2 * FSH : 3 * FSH],
                )
            for k in range(KSUB):
                nc.sync.dma_start(
                    out=wqkv_sb[:, k, FSH : 2 * FSH],
                    in_=wqkv_r[:, k, FSH : 2 * FSH],
                )
            for p_ in range(NPAIR):
                nc.vector.memset(lhsT2_sb[p_], 0.0)
                nc.vector.memset(ksumrep_sb[p_], 0.0)
            for j in range(1, NBLK):
                nc.sync.dma_start(
                    out=xt_sb[:, :, j * SBLK : (j + 1) * SBLK],
                    in_=xT_r[:, :, j * SBLK : (j + 1) * SBLK],
                )
            for k in range(KSUB):
                nc.sync.dma_start(
                    out=wqkv_sb[:, k, 0:FSH], in_=wqkv_r[:, k, 0:FSH]
                )
            nc.sync.dma_start(out=wo_sb, in_=wo_r)

            # SBUF pools shared across both phases
            etpool = ctx.enter_context(tc.tile_pool(name="et", bufs=3))
            qtpool = ctx.enter_context(tc.tile_pool(name="qt", bufs=2))
            qts = {}

            def qt_elu(ps, j, f):
                # elu(x)+1 = min(exp(x),1) + relu(x); Exp/Relu on ACT, the
                # combine on DVE
                e = etpool.tile([P, SBLK], F32, tag="e")
                nc.scalar.activation(out=e, in_=ps, func=ACT.Exp)
                r = etpool.tile([P, SBLK], F32, tag="r")
                nc.scalar.activation(out=r, in_=ps, func=ACT.Relu)
                nc.vector.scalar_tensor_tensor(
                    out=qts[j][:, f, :],
                    in0=e,
                    scalar=1.0,
                    in1=r,
                    op0=mybir.AluOpType.min,
                    op1=mybir.AluOpType.add,
                )

            # ---------------- phase 1: K,V projection + KV accumulation ----
            with (
                tc.tile_pool(name="kvps", bufs=1, space="PSUM") as kvps_pool,
                tc.tile_pool(name="pa", bufs=4, space="PSUM") as pa_pool,
                tc.tile_pool(name="st", bufs=2) as stpool,
            ):
                kvps = [
                    kvps_pool.tile([P, P + 1], F32, tag=f"kv{p}", name=f"kv{p}")
                    for p in range(NPAIR)
                ]

                bq = []  # lagged [KV | K_sum] accumulation entries

                def emit_b(ent):
                    kst, vst, j, t = ent
                    first = j == 0 and t == 0
                    last = j == NBLK - 1 and t == TSUB - 1
                    for p_ in range(NPAIR):
                        nc.tensor.matmul(
                            kvps[p_],
                            kst[:, t, p_ * P : (p_ + 1) * P],
                            vst[:, t, p_, :],
                            start=first,
                            stop=last,
                        )

                def elu_k(ps, kst, t):
                    e = etpool.tile([P, SBLK], F32, tag="e")
                    nc.scalar.activation(out=e, in_=ps, func=ACT.Exp)
                    r = etpool.tile([P, SBLK], F32, tag="r")
                    nc.vector.tensor_scalar_max(r, ps, 0.0)
                    nc.vector.scalar_tensor_tensor(
                        out=kst[:, t, :],
                        in0=e,
                        scalar=1.0,
                        in1=r,
                        op0=mybir.AluOpType.min,
                        op1=mybir.AluOpType.add,
                    )

                # block 0: V-sweep k-outer so PE work tracks DMA chunk
                # arrival (V needs no activation, so the K sweep that
                # follows runs at PE speed with elu pipelined per subtile)
                kst0 = stpool.tile([P, TSUB, FSH], BF16, tag="kst")
                vst0 = stpool.tile([P, TSUB, NPAIR, P + 1], BF16, tag="vst")
                nc.vector.memset(vst0[:, :, :, P : P + 1], 1.0)
                psvs = [
                    pa_pool.tile([P, SBLK], F32, tag="pa", name=f"psv{t}")
                    for t in range(TSUB)
                ]
                for k in range(KSUB):
                    for t in range(TSUB):
                        nc.tensor.matmul(
                            psvs[t],
                            xt_sb[:, k, t * P : (t + 1) * P],
                            wqkv_sb[:, k, 2 * FSH : 3 * FSH],
                            start=(k == 0),
                            stop=(k == KSUB - 1),
                        )
                for t in range(TSUB):
                    nc.scalar.copy(out=vst0[:, t, :, 0:P], in_=psvs[t])
                for t in range(TSUB):
                    psk = pa_pool.tile([P, SBLK], F32, tag="pa", name=f"psk{t}")
                    for k in range(KSUB):
                        nc.tensor.matmul(
                            psk,
                            xt_sb[:, k, t * P : (t + 1) * P],
                            wqkv_sb[:, k, FSH : 2 * FSH],
                            start=(k == 0),
                            stop=(k == KSUB - 1),
                        )
                    if t >= 1:
                        emit_b(bq.pop(0))
                    elu_k(psk, kst0, t)
                    bq.append((kst0, vst0, 0, t))

                # blocks 1..7: token-subtile-outer, B lagged one step
                for j in range(1, NBLK):
                    kst = stpool.tile([P, TSUB, FSH], BF16, tag="kst")
                    vst = stpool.tile([P, TSUB, NPAIR, P + 1], BF16, tag="vst")
                    nc.vector.memset(vst[:, :, :, P : P + 1], 1.0)
                    for t in range(TSUB):
                        tok = j * SBLK + t * P
                        psk = pa_pool.tile([P, SBLK], F32, tag="pa")
                        psv = pa_pool.tile([P, SBLK], F32, tag="pa")
                        for k in range(KSUB):
                            nc.tensor.matmul(
                                psk,
                                xt_sb[:, k, tok : tok + P],
                                wqkv_sb[:, k, FSH : 2 * FSH],
                                start=(k == 0),
                                stop=(k == KSUB - 1),
                            )
                            nc.tensor.matmul(
                                psv,
                                xt_sb[:, k, tok : tok + P],
                                wqkv_sb[:, k, 2 * FSH : 3 * FSH],
                                start=(k == 0),
                                stop=(k == KSUB - 1),
                            )
                        emit_b(bq.pop(0))
                        elu_k(psk, kst, t)
                        nc.scalar.copy(out=vst[:, t, :, 0:P], in_=psv)
                        bq.append((kst, vst, j, t))
                # block 0's Q projection runs here, inside the phase-1 PSUM
                # pools: it has no dependency on the KV state, so it keeps
                # the PE busy across the phase boundary (the trailing elu,
                # the KV extraction, and the phase-2 pool handover all hide
                # under its 32 matmuls)
                qts[0] = qtpool.tile([P, NPAIR, SBLK], BF16, tag="qt", name="qt0")
                for f in range(FSH // P):
                    psq = pa_pool.tile([P, SBLK], F32, tag="pa")
                    for k in range(KSUB):
                        nc.tensor.matmul(
                            psq,
                            wqkv_sb[:, k, f * P : (f + 1) * P],
                            xt_sb[:, k, 0:SBLK],
                            start=(k == 0),
                            stop=(k == KSUB - 1),
                        )
                    if f == 0:
                        emit_b(bq.pop(0))
                    qt_elu(psq, 0, f)

                # extraction: blockdiag KV + replicated K_sum (zeros preset)
                for p_ in range(NPAIR):
                    nc.vector.tensor_copy(
                        out=lhsT2_sb[p_][0:HD, 0:HD], in_=kvps[p_][0:HD, 0:HD]
                    )
                    nc.vector.tensor_copy(
                        out=lhsT2_sb[p_][HD:P, HD:P], in_=kvps[p_][HD:P, HD:P]
                    )
                    nc.vector.tensor_copy(
                        out=ksumrep_sb[p_][0:HD, 0:HD],
                        in_=kvps[p_][0:HD, P : P + 1].to_broadcast((HD, HD)),
                    )
                    nc.vector.tensor_copy(
                        out=ksumrep_sb[p_][HD:P, HD:P],
                        in_=kvps[p_][HD:P, P : P + 1].to_broadcast((HD, HD)),
                    )

            # ---------------- phase 2: Q projection + attention + Wo -------
            with (
                tc.tile_pool(name="mm512", bufs=3, space="PSUM") as mmps,
                tc.tile_pool(name="pc", bufs=3, space="PSUM") as pcps,
                tc.tile_pool(name="pnb", bufs=2, space="PSUM") as pnps,
                tc.tile_pool(name="ou", bufs=2) as oupool,
                tc.tile_pool(name="rc", bufs=4) as rcpool,
                tc.tile_pool(name="ot", bufs=2) as otpool,
                tc.tile_pool(name="ys", bufs=2) as ypool,
            ):
                outus = {}
                rcbs = {}
                outts = {}

                def qt_half(j, fh):
                    if j not in qts:
                        qts[j] = qtpool.tile(
                            [P, NPAIR, SBLK], BF16, tag="qt", name=f"qt{j}"
                        )
                    for f in (2 * fh, 2 * fh + 1):
                        ps = mmps.tile([P, SBLK], F32, tag="mm")
                        for k in range(KSUB):
                            nc.tensor.matmul(
                                ps,
                                wqkv_sb[:, k, f * P : (f + 1) * P],
                                xt_sb[:, k, j * SBLK : (j + 1) * SBLK],
                                start=(k == 0),
                                stop=(k == KSUB - 1),
                            )
                        qt_elu(ps, j, f)

                def psc_section(j):
                    # per pair: attention matmul (ACT-evicted) + broadcast
                    # normalizer matmul (DVE fast reciprocal, stays in SBUF)
                    qtj = qts.pop(j)
                    outu = oupool.tile([P, NPAIR, SBLK], F32, tag="outu")
                    outus[j] = outu
                    rcbs[j] = []
                    for p_ in range(NPAIR):
                        psc = pcps.tile([P, SBLK], F32, tag="pc")
                        nc.tensor.matmul(
                            psc,
                            lhsT2_sb[p_],
                            qtj[:, p_, :],
                            start=True,
                            stop=True,
                        )
                        nc.scalar.copy(out=outu[:, p_, :], in_=psc)
                        psn = pnps.tile([P, SBLK], F32, tag="pn")
                        nc.tensor.matmul(
                            psn,
                            ksumrep_sb[p_],
                            qtj[:, p_, :],
                            start=True,
                            stop=True,
                        )
                        rcb = rcpool.tile([P, SBLK], F32, tag="rcb")
                        nc.vector.reciprocal_approx_fast(out=rcb[:], in_=psn[:])
                        rcbs[j].append(rcb)

                def mults(j):
                    outt = otpool.tile([P, NPAIR, SBLK], BF16, tag="outt")
                    outts[j] = outt
                    outu = outus.pop(j)
                    rcs = rcbs.pop(j)
                    for p_ in range(NPAIR):
                        nc.vector.tensor_tensor(
                            out=outt[:, p_, :],
                            in0=outu[:, p_, :],
                            in1=rcs[p_],
                            op=mybir.AluOpType.mult,
                        )

                def d_t(j, outt, t):
                    ysb = ypool.tile([P, D], F32, tag="ysb", name="ysb")
                    psy0 = mmps.tile([P, 512], F32, tag="mm", name="psy0")
                    psy1 = mmps.tile([P, 512], F32, tag="mm", name="psy1")
                    for fs in range(FSH // P):
                        nc.tensor.matmul(
                            psy0,
                            outt[:, fs, t * P : (t + 1) * P],
                            wo_sb[:, fs, 0:512],
                            start=(fs == 0),
                            stop=(fs == FSH // P - 1),
                        )
                        nc.tensor.matmul(
                            psy1,
                            outt[:, fs, t * P : (t + 1) * P],
                            wo_sb[:, fs, 512:1024],
                            start=(fs == 0),
                            stop=(fs == FSH // P - 1),
                        )
                    nc.scalar.copy(out=ysb[:, 0:512], in_=psy0)
                    nc.sync.dma_start(out=y_rt[j, t, 0], in_=ysb[:, 0:512])
                    nc.vector.tensor_copy(out=ysb[:, 512:1024], in_=psy1)
                    nc.sync.dma_start(out=y_rt[j, t, 1], in_=ysb[:, 512:1024])

                def d_block(j):
                    outt = outts.pop(j)
                    for t in range(TSUB):
                        d_t(j, outt, t)

                def finale(j):
                    # drain block: apply-multiplies split per token subtile
                    # so each D chain starts as soon as its slice is scaled
                    outt = otpool.tile([P, NPAIR, SBLK], BF16, tag="outt")
                    outu = outus.pop(j)
                    rcs = rcbs.pop(j)
                    for t in range(TSUB):
                        sl = slice(t * P, (t + 1) * P)
                        for p_ in range(NPAIR):
                            nc.vector.tensor_tensor(
                                out=outt[:, p_, sl],
                                in0=outu[:, p_, sl],
                                in1=rcs[p_][:, sl],
                                op=mybir.AluOpType.mult,
                            )
                        d_t(j, outt, t)

                # steady-state emission: block j's Q projection brackets
                # block j-1's attention chain so the PE never waits on the
                # ACT/DVE eviction+reciprocal+apply latency.
                for j in range(1, NBLK):
                    psc_section(j - 1)
                    mults(j - 1)
                    qt_half(j, 0)
                    qt_half(j, 1)
                    d_block(j - 1)
                psc_section(NBLK - 1)
                finale(NBLK - 1)

    nc.compile()
    return nc


def _prep_inputs(x, Wqkv, Wo):
    import ml_dtypes

    x = np.ascontiguousarray(x, dtype=np.float32)
    Wqkv = np.ascontiguousarray(Wqkv, dtype=np.float32)
    Wo = np.ascontiguousarray(Wo, dtype=np.float32)
    in_maps = []
    for b in range(B):
        xT = np.ascontiguousarray(x[b].T).astype(ml_dtypes.bfloat16)  # [D, S]
        for hh in range(2):
            cols = slice(hh * FSH, (hh + 1) * FSH)
            wq = Wqkv[:, 0 * D :][:, cols]
            wk = Wqkv[:, 1 * D :][:, cols]
            wv = Wqkv[:, 2 * D :][:, cols]
            wqkv_sh = np.ascontiguousarray(
                np.concatenate([wq, wk, wv], axis=1)
            ).astype(ml_dtypes.bfloat16)
            wo_sh = np.ascontiguousarray(Wo[hh * FSH : (hh + 1) * FSH, :]).astype(
                ml_dtypes.bfloat16
            )
            in_maps.append({"xT": xT, "wqkv": wqkv_sh, "wo": wo_sh})
    return in_maps


def kernel(x, Wqkv, Wo):
    global _NC_CACHE
    if _NC_CACHE is None:
        _NC_CACHE = build()
    nc = _NC_CACHE
    in_maps = _prep_inputs(x, Wqkv, Wo)
    res = run_bass_kernel_spmd(nc, in_maps, list(range(2 * B))).results
    y = np.empty((B, S, D), dtype=np.float32)
    for b in range(B):
        y[b] = res[2 * b]["y"] + res[2 * b + 1]["y"]
    return y
